# revision 4
# baseline (speedup 1.0000x reference)
"""Trainium2 Bass kernel for nn_CrowdHumanPostProcess (B=64, N=100000, C=1).

Sharding: pure data parallel — batch rows 8 per core across 8 NeuronCores.

Device pipeline per row:
  - Bit-exact replication of XLA:CPU's sigmoid (FMA-contracted Cephes exp)
    on the Vector engine via Dekker/TwoSum exact-FMA emulation, so sort-key
    ties match the CPU reference exactly.
  - Full descending stable sort of the 100000 keys: bitonic network over
    (key, index) pairs on a [128 x 1024] padded layout; DVE 32x32 stream
    transposes switch between layouts so all compare-exchanges are
    free-dimension ops; odd-even tie-repair passes restore index-ascending
    order within equal keys (plus a rotated pass for partition boundaries).
  - Scores = sorted keys; sorted indices are returned for the box gather.

Host: assembles outputs; labels are the constant 1; box cxcywh->xyxy
transform + scale + gather currently on host (device port pending).

Self-contained: hardcodes shapes; no sibling imports.
"""
import struct
import numpy as np
from contextlib import ExitStack

import concourse.bass as bass
import concourse.mybir as mybir
from concourse.bass_utils import run_bass_kernel_spmd

A = mybir.AluOpType
F32 = mybir.dt.float32
I32 = mybir.dt.int32

B, N, C = 64, 100000, 1
NCORES = 8
RPC = B // NCORES           # rows per core
FW = 1024
NE = 128 * FW
FB = 10
NB = 17
SEG = 784                   # 128*784 = 100352 padded row length
NPAD = 128 * SEG
XPAD = np.float32(-30.0)    # sigmoid(-30) ~ 9.36e-14, below all real keys
REPAIR_PASSES = 8


def _dh(h):
    return np.float32(struct.unpack(">d", bytes.fromhex(h))[0])


LOG2E = _dh("3FF7154760000000")
CW1 = _dh("3FE6300000000000")
C2P = np.float32(-_dh("BF2BD01060000000"))
P0 = _dh("3F2A0D2CE0000000")
P1 = _dh("3F56E879C0000000")
P2 = _dh("3F81112100000000")
P3 = _dh("3FA5553820000000")
P4 = _dh("3FC5555540000000")
HALF = np.float32(0.5)


def _split12(c):
    bits = np.float32(c).view(np.uint32)
    hi = np.uint32(bits & np.uint32(0xFFFFF000)).view(np.float32)
    lo = np.float32(np.float32(c) - hi)
    return np.float32(hi), np.float32(lo)


C2H, C2L = _split12(C2P)
P0H, P0L = _split12(P0)
MAGIC = np.float32(12582912.0)


def emit_sigmoid(nc, eng, x, out, sc, sci):
    """out[:, 0:SEG] = bitexact XLA-CPU sigmoid of x[:, 0:SEG]."""
    t_, q_, m_, r1_, r_, rh_, rl_, p_, ph_, pl_, e_, s_, w0, w1 = sc[:14]
    sl = np.s_[:, 0:SEG]
    S = lambda t: t[sl]

    def ts(o, i, s1, op0, s2=None, op1=None):
        if s2 is None:
            eng.tensor_scalar(out=S(o), in0=S(i), scalar1=float(s1),
                              scalar2=None, op0=op0)
        else:
            eng.tensor_scalar(out=S(o), in0=S(i), scalar1=float(s1),
                              scalar2=float(s2), op0=op0, op1=op1)

    def tt(o, a, b, op):
        eng.tensor_tensor(out=S(o), in0=S(a), in1=S(b), op=op)

    ts(t_, x, -1.0, A.mult)
    ts(q_, t_, LOG2E, A.mult, HALF, A.add)
    ts(m_, q_, MAGIC, A.add, MAGIC, A.subtract)
    tt(w0, m_, q_, A.is_gt)
    tt(m_, m_, w0, A.subtract)
    ts(w0, m_, CW1, A.mult)
    tt(r1_, t_, w0, A.subtract)
    ts(w0, m_, C2H, A.mult)
    ts(w1, m_, C2L, A.mult)
    tt(s_, r1_, w0, A.add)
    tt(e_, s_, r1_, A.subtract)
    tt(rh_, s_, e_, A.subtract)
    tt(rl_, r1_, rh_, A.subtract)
    tt(e_, w0, e_, A.subtract)
    tt(e_, rl_, e_, A.add)
    tt(e_, e_, w1, A.add)
    tt(r_, s_, e_, A.add)
    ts(w0, r_, 4097.0, A.mult)
    tt(w1, w0, r_, A.subtract)
    tt(rh_, w0, w1, A.subtract)
    tt(rl_, r_, rh_, A.subtract)
    ts(ph_, r_, P0, A.mult)
    ts(e_, rh_, P0H, A.mult)
    tt(e_, e_, ph_, A.subtract)
    ts(w0, rh_, P0L, A.mult); tt(e_, e_, w0, A.add)
    ts(w0, rl_, P0H, A.mult); tt(e_, e_, w0, A.add)
    ts(w0, rl_, P0L, A.mult); tt(e_, e_, w0, A.add)
    ts(s_, ph_, P1, A.add)
    ts(w0, s_, P1, A.subtract)
    tt(w1, ph_, w0, A.subtract)
    tt(e_, w1, e_, A.add)
    tt(p_, s_, e_, A.add)
    for K in (P2, P3, P4, HALF):
        ts(w0, p_, 4097.0, A.mult)
        tt(w1, w0, p_, A.subtract)
        tt(ph_, w0, w1, A.subtract)
        tt(pl_, p_, ph_, A.subtract)
        tt(t_, p_, r_, A.mult)
        tt(e_, ph_, rh_, A.mult)
        tt(e_, e_, t_, A.subtract)
        tt(w0, ph_, rl_, A.mult); tt(e_, e_, w0, A.add)
        tt(w0, pl_, rh_, A.mult); tt(e_, e_, w0, A.add)
        tt(w0, pl_, rl_, A.mult); tt(e_, e_, w0, A.add)
        ts(s_, t_, K, A.add)
        ts(w0, s_, K, A.subtract)
        tt(w1, t_, w0, A.subtract)
        tt(e_, w1, e_, A.add)
        tt(p_, s_, e_, A.add)
    tt(q_, r_, r_, A.mult)
    ts(w0, p_, 4097.0, A.mult)
    tt(w1, w0, p_, A.subtract)
    tt(ph_, w0, w1, A.subtract)
    tt(pl_, p_, ph_, A.subtract)
    ts(w0, q_, 4097.0, A.mult)
    tt(w1, w0, q_, A.subtract)
    tt(rh_, w0, w1, A.subtract)
    tt(rl_, q_, rh_, A.subtract)
    tt(t_, p_, q_, A.mult)
    tt(e_, ph_, rh_, A.mult)
    tt(e_, e_, t_, A.subtract)
    tt(w0, ph_, rl_, A.mult); tt(e_, e_, w0, A.add)
    tt(w0, pl_, rh_, A.mult); tt(e_, e_, w0, A.add)
    tt(w0, pl_, rl_, A.mult); tt(e_, e_, w0, A.add)
    tt(s_, r_, t_, A.add)
    tt(w0, s_, r_, A.subtract)
    tt(w1, s_, w0, A.subtract)
    tt(r1_, r_, w1, A.subtract)
    tt(w1, t_, w0, A.subtract)
    tt(r1_, r1_, w1, A.add)
    tt(e_, r1_, e_, A.add)
    tt(s_, s_, e_, A.add)
    ts(s_, s_, 1.0, A.add)
    ts(w0, m_, 127.0, A.add, 8388608.0, A.mult)
    eng.tensor_copy(out=S(sci), in_=S(w0))
    tt(w1, s_, sci[sl].bitcast(F32), A.mult)
    ts(w1, w1, 1.0, A.add)
    eng.reciprocal(out=S(out), in_=S(w1))


def sort_stage_list():
    plan = []
    for k in range(1, NB + 1):
        ds_ = [2 ** j for j in range(k - 1, -1, -1)]
        big = [d for d in ds_ if d >= FW]
        small = [d for d in ds_ if d <= FW // 2]
        if big:
            plan.append(("T12", k))
            for d in big:
                plan.append(("A2", k, d))
            plan.append(("T21", k))
        for d in small:
            plan.append(("A", k, d) if k <= FB - 1 else ("B", k, d))
    return plan


def _ap(tile, offset, dims, dtype=None):
    ap = bass.AP(tile[:].tensor, offset, [list(x) for x in dims])
    return ap.bitcast(dtype) if dtype is not None else ap


def _pair_dims(k_dirbit, ld):
    d = 2 ** ld
    dims = []
    if k_dirbit is not None:
        if 2 ** (FB - 1 - k_dirbit) > 1:
            dims.append((2 ** (k_dirbit + 1), 2 ** (FB - 1 - k_dirbit)))
        if 2 ** (k_dirbit - 1 - ld) > 1:
            dims.append((2 * d, 2 ** (k_dirbit - 1 - ld)))
    else:
        if 2 ** (FB - 1 - ld) > 1:
            dims.append((2 * d, 2 ** (FB - 1 - ld)))
    dims.append((1, d))
    return dims


class SortCtx:
    def __init__(self, nc, eng, K, I, K2, I2, g_f, g_f2, g_i, dircols):
        self.nc, self.eng = nc, eng
        self.K, self.I, self.K2, self.I2 = K, I, K2, I2
        self.g_f, self.g_f2, self.g_i = g_f, g_f2, g_i
        self.dircols = dircols

    def flip(self):
        self.K, self.K2 = self.K2, self.K
        self.I, self.I2 = self.I2, self.I


def emit_stage_A(ctx, k_dirbit, ld):
    eng = ctx.eng
    d = 2 ** ld
    dims = _pair_dims(k_dirbit, ld)
    sets = [(0, True)] if k_dirbit is None else [(0, True), (2 ** k_dirbit, False)]
    for off, desc in sets:
        APf = lambda t, ex=0: _ap(t, off + ex, [(FW, 128)] + dims)
        Ka, Kb = APf(ctx.K), APf(ctx.K, d)
        Ia, Ib = APf(ctx.I), APf(ctx.I, d)
        KoA, KoB = APf(ctx.K2), APf(ctx.K2, d)
        IoA, IoB = APf(ctx.I2), APf(ctx.I2, d)
        W = _ap(ctx.g_f, off, [(FW, 128)] + dims)
        D_ = _ap(ctx.g_f2, off, [(FW, 128)] + dims)
        eng.tensor_tensor(out=KoA, in0=Ka, in1=Kb, op=A.max if desc else A.min)
        eng.tensor_tensor(out=KoB, in0=Ka, in1=Kb, op=A.min if desc else A.max)
        eng.tensor_tensor(out=W, in0=Ka, in1=Kb, op=A.is_ge if desc else A.is_le)
        eng.tensor_tensor(out=D_, in0=Ia, in1=Ib, op=A.subtract)
        eng.tensor_tensor(out=D_, in0=W, in1=D_, op=A.mult)
        eng.tensor_tensor(out=IoA, in0=Ib, in1=D_, op=A.add)
        eng.tensor_tensor(out=IoB, in0=Ia, in1=D_, op=A.subtract)
    ctx.flip()


def emit_stage_B(ctx, k, ld):
    eng = ctx.eng
    d = 2 ** ld
    dims = _pair_dims(None, ld)
    APf = lambda t, ex=0, dt=None: _ap(t, ex, [(FW, 128)] + dims, dt)
    Ka, Kb = APf(ctx.K), APf(ctx.K, d)
    Kai, Kbi = APf(ctx.K, 0, I32), APf(ctx.K, d, I32)
    Iai, Ibi = APf(ctx.I, 0, I32), APf(ctx.I, d, I32)
    KoAi, KoBi = APf(ctx.K2, 0, I32), APf(ctx.K2, d, I32)
    IoAi, IoBi = APf(ctx.I2, 0, I32), APf(ctx.I2, d, I32)
    gi, Mi, x_ = APf(ctx.g_i[0]), APf(ctx.g_i[1]), APf(ctx.g_i[2])
    dc = ctx.dircols[:, min(k - FB, 7):min(k - FB, 7) + 1]
    eng.tensor_tensor(out=gi, in0=Ka, in1=Kb, op=A.is_ge)
    eng.tensor_scalar(out=Mi, in0=gi, scalar1=dc, scalar2=None, op0=A.is_equal)
    eng.tensor_scalar(out=Mi, in0=Mi, scalar1=31, scalar2=31,
                      op0=A.logical_shift_left, op1=A.arith_shift_right)
    eng.tensor_tensor(out=x_, in0=Kai, in1=Kbi, op=A.bitwise_xor)
    eng.tensor_tensor(out=x_, in0=x_, in1=Mi, op=A.bitwise_and)
    eng.tensor_tensor(out=KoAi, in0=Kbi, in1=x_, op=A.bitwise_xor)
    eng.tensor_tensor(out=KoBi, in0=Kai, in1=x_, op=A.bitwise_xor)
    eng.tensor_tensor(out=x_, in0=Iai, in1=Ibi, op=A.bitwise_xor)
    eng.tensor_tensor(out=x_, in0=x_, in1=Mi, op=A.bitwise_and)
    eng.tensor_tensor(out=IoAi, in0=Ibi, in1=x_, op=A.bitwise_xor)
    eng.tensor_tensor(out=IoBi, in0=Iai, in1=x_, op=A.bitwise_xor)
    ctx.flip()


def emit_transpose(ctx, to_l2):
    eng = ctx.eng
    G = FW // 128
    for src, dst in ((ctx.K, ctx.K2), (ctx.I, ctx.I2)):
        l1_t = src if to_l2 else dst
        l2_t = dst if to_l2 else src
        for i in range(4):
            for j in range(4):
                l1 = bass.AP(l1_t[:].tensor, 32 * j * FW + 32 * i,
                             [[FW, 32], [128, G], [1, 32]])
                l2 = bass.AP(l2_t[:].tensor, 32 * i * FW + G * 32 * j,
                             [[FW, 32], [1, G], [G, 32]])
                if to_l2:
                    eng.transpose(out=l2, in_=l1)
                else:
                    eng.transpose(out=l1, in_=l2)
    ctx.flip()


def emit_sort(ctx):
    for st in sort_stage_list():
        if st[0] == "T12":
            emit_transpose(ctx, True)
        elif st[0] == "T21":
            emit_transpose(ctx, False)
        elif st[0] == "A":
            emit_stage_A(ctx, st[1], int(np.log2(st[2])))
        elif st[0] == "A2":
            kk = st[1] - 7
            emit_stage_A(ctx, kk if kk <= FB - 1 else None, int(np.log2(st[2] // 128)))
        else:
            emit_stage_B(ctx, st[1], int(np.log2(st[2])))


def emit_repair(ctx, K, I, passes):
    eng = ctx.eng
    for ph in range(passes):
        off = ph % 2
        n = (FW - off) // 2
        APf = lambda t, ex=0: _ap(t, off + ex, [(FW, 128), (2, n)])
        Ka, Kb = APf(K), APf(K, 1)
        Ia, Ib = APf(I), APf(I, 1)
        W = _ap(ctx.g_f, off, [(FW, 128), (2, n)])
        D_ = _ap(ctx.g_f2, off, [(FW, 128), (2, n)])
        eng.tensor_tensor(out=W, in0=Ka, in1=Kb, op=A.is_equal)
        eng.tensor_tensor(out=D_, in0=Ia, in1=Ib, op=A.is_gt)
        eng.tensor_tensor(out=W, in0=W, in1=D_, op=A.mult)
        eng.tensor_tensor(out=D_, in0=Ia, in1=Ib, op=A.subtract)
        eng.tensor_tensor(out=D_, in0=W, in1=D_, op=A.mult)
        eng.tensor_tensor(out=Ia, in0=Ia, in1=D_, op=A.subtract)
        eng.tensor_tensor(out=Ib, in0=Ib, in1=D_, op=A.add)


def host_dircols():
    p = np.arange(128)
    cols = np.zeros((128, 8), np.float32)
    for j in range(8):
        k = FB + j
        cols[:, j] = 1.0 if k >= NB else ((p >> (k - FB)) & 1 == 0).astype(np.float32)
    return cols


_CACHE = {}


def _build_program():
    nc = bass.Bass(detect_race_conditions=False)
    x_in = nc.declare_dram_parameter("x", [RPC, NPAD], F32, isOutput=False)
    dcol_in = nc.declare_dram_parameter("dcols", [128, 8], F32, isOutput=False)
    sco_out = nc.declare_dram_parameter("scores", [RPC, 128, FW], F32, isOutput=True)
    idx_out = nc.declare_dram_parameter("sidx", [RPC, 128, FW], I32, isOutput=True)

    with ExitStack() as ctx:
        XT = ctx.enter_context(nc.sbuf_tensor("XT", [128, SEG], F32))
        K = ctx.enter_context(nc.sbuf_tensor("K", [128, FW], F32))
        I = ctx.enter_context(nc.sbuf_tensor("I", [128, FW], F32))
        K2 = ctx.enter_context(nc.sbuf_tensor("K2", [128, FW], F32))
        I2 = ctx.enter_context(nc.sbuf_tensor("I2", [128, FW], F32))
        KR = ctx.enter_context(nc.sbuf_tensor("KR", [128, FW], F32))
        IR = ctx.enter_context(nc.sbuf_tensor("IR", [128, FW], F32))
        g_f = ctx.enter_context(nc.sbuf_tensor("g_f", [128, FW], F32))
        g_f2 = ctx.enter_context(nc.sbuf_tensor("g_f2", [128, FW], F32))
        g_i = [ctx.enter_context(nc.sbuf_tensor(f"g_i{i}", [128, FW], I32)) for i in range(3)]
        IOTA = ctx.enter_context(nc.sbuf_tensor("IOTA", [128, FW], I32))
        IXI = ctx.enter_context(nc.sbuf_tensor("IXI", [128, FW], I32))
        sc = [ctx.enter_context(nc.sbuf_tensor(f"sc{i}", [128, SEG], F32)) for i in range(14)]
        sci = ctx.enter_context(nc.sbuf_tensor("sci", [128, SEG], I32))
        dcols = ctx.enter_context(nc.sbuf_tensor("dcols_sb", [128, 8], F32))
        dma = ctx.enter_context(nc.semaphore("dma"))
        gdma = ctx.enter_context(nc.semaphore("gdma"))
        vs = ctx.enter_context(nc.semaphore("vs"))
        vs2 = ctx.enter_context(nc.semaphore("vs2"))
        gs = ctx.enter_context(nc.semaphore("gs"))
        dsm = ctx.enter_context(nc.semaphore("dsm"))
        block = ctx.enter_context(nc.Block())

        sctx = SortCtx(nc, nc.vector, K, I, K2, I2, g_f, g_f2, g_i, dcols)
        rot_tiles = {}
        fin_tiles = {}

        @block.vector
        def _(v):
            for r in range(RPC):
                v.wait_ge(dsm, 2 * r + 1)
                nc.vector.memset(sctx.K[:, SEG:FW], -1.0)
                emit_sigmoid(nc, nc.vector, XT, sctx.K, sc, sci)
                nc.vector.sem_inc(vs, 1)            # -> 2r+1: XT consumed
                nc.vector.tensor_copy(out=sctx.I[:], in_=IOTA[:])  # int->f32
                emit_sort(sctx)
                emit_repair(sctx, sctx.K, sctx.I, REPAIR_PASSES)
                rot_tiles[r] = (sctx.K, sctx.I)
                nc.vector.sem_inc(vs2, 1)           # -> 2r+1: rotate-in OK
                v.wait_ge(gs, 2 * r + 1)
                emit_repair(sctx, KR, IR, REPAIR_PASSES)
                nc.vector.sem_inc(vs2, 1)           # -> 2r+2: rotate-out OK
                v.wait_ge(gs, 2 * r + 2)
                nc.vector.tensor_copy(out=IXI[:], in_=sctx.I[:])   # f32 -> int32
                fin_tiles[r] = sctx.K
                nc.vector.sem_inc(vs, 1)            # -> 2r+2: row done

        @block.gpsimd
        def _(g):
            # constant index payload (same every row): 0..100351 then tail ids
            nc.gpsimd.iota(IOTA[:, 0:SEG], pattern=[[1, SEG]], base=0,
                           channel_multiplier=SEG)
            nc.gpsimd.iota(IOTA[:, SEG:FW], pattern=[[1, FW - SEG]], base=NPAD,
                           channel_multiplier=FW - SEG)
            nc.gpsimd.memset(KR[:], -2.0)
            nc.gpsimd.memset(IR[:], 0.0)
            nd = [0]

            def gd(dst, src):
                nc.gpsimd.dma_start(out=dst, in_=src).then_inc(gdma, 16)
                nd[0] += 16

            for r in range(RPC):
                g.wait_ge(vs2, 2 * r + 1)
                Kt, It = rot_tiles[r]
                gd(KR[:, 0:512], Kt[:, 512:FW])
                gd(bass.AP(KR[:].tensor, 512, [[FW, 127], [1, 512]]),
                   bass.AP(Kt[:].tensor, FW, [[FW, 127], [1, 512]]))
                gd(IR[:, 0:512], It[:, 512:FW])
                gd(bass.AP(IR[:].tensor, 512, [[FW, 127], [1, 512]]),
                   bass.AP(It[:].tensor, FW, [[FW, 127], [1, 512]]))
                g.wait_ge(gdma, nd[0])
                nc.gpsimd.sem_inc(gs, 1)            # -> 2r+1
                g.wait_ge(vs2, 2 * r + 2)
                gd(It[:, 512:FW], IR[:, 0:512])
                gd(bass.AP(It[:].tensor, FW, [[FW, 127], [1, 512]]),
                   bass.AP(IR[:].tensor, 512, [[FW, 127], [1, 512]]))
                g.wait_ge(gdma, nd[0])
                nc.gpsimd.sem_inc(gs, 1)            # -> 2r+2

        @block.sync
        def _(s):
            nd = [0]

            def sd(dst, src):
                nc.sync.dma_start(out=dst, in_=src).then_inc(dma, 16)
                nd[0] += 16

            sd(dcols[:], dcol_in[:])
            for r in range(RPC):
                sd(XT[:], bass.AP(x_in[:].tensor, r * NPAD, [[SEG, 128], [1, SEG]]))
                s.wait_ge(dma, nd[0])
                nc.sync.sem_inc(dsm, 1)             # -> 2r+1: input ready
                s.wait_ge(vs, 2 * r + 2)            # row done
                sd(sco_out[r, :, :], fin_tiles[r][:])
                sd(idx_out[r, :, :], IXI[:])
                s.wait_ge(dma, nd[0])
                nc.sync.sem_inc(dsm, 1)             # -> 2r+2: flushed

    return nc


def kernel(pred_logits, pred_boxes, target_sizes):
    x = np.ascontiguousarray(pred_logits[:, :, 0], dtype=np.float32)
    xpad = np.full((B, NPAD), XPAD, np.float32)
    xpad[:, :N] = x
    if "nc" not in _CACHE:
        _CACHE["nc"] = _build_program()
    nc = _CACHE["nc"]
    dcols = host_dircols()
    in_maps = [{"x": np.ascontiguousarray(xpad[c * RPC:(c + 1) * RPC]),
                "dcols": dcols} for c in range(NCORES)]
    results = run_bass_kernel_spmd(nc, in_maps, list(range(NCORES))).results

    scores = np.empty((B, N), np.float32)
    topk = np.empty((B, N), np.int64)
    for c in range(NCORES):
        sco = results[c]["scores"].reshape(RPC, NE)
        six = results[c]["sidx"].reshape(RPC, NE)
        scores[c * RPC:(c + 1) * RPC] = sco[:, :N]
        topk[c * RPC:(c + 1) * RPC] = six[:, :N]

    labels = np.ones((B, N), np.int32)

    # boxes: cxcywh -> xyxy, scale, gather by sorted index
    # TODO(device): move transform+gather onto the NeuronCores.
    b = pred_boxes.astype(np.float32)
    cx, cy, w, h = b[..., 0], b[..., 1], b[..., 2], b[..., 3]
    hw = (np.float32(0.5) * w).astype(np.float32)
    hh = (np.float32(0.5) * h).astype(np.float32)
    boxes = np.stack([cx - hw, cy - hh, cx + hw, cy + hh], axis=-1).astype(np.float32)
    img_h = target_sizes[:, 0].astype(np.float32)
    img_w = target_sizes[:, 1].astype(np.float32)
    scale = np.stack([img_w, img_h, img_w, img_h], axis=1).astype(np.float32)
    boxes = (boxes * scale[:, None, :]).astype(np.float32)
    boxes = np.take_along_axis(boxes, topk[..., None], axis=1)

    return scores, labels, boxes


def np_sigmoid_exact_host(x):
    """Host emulator of the exact device/XLA-CPU sigmoid (verified bit-exact)."""
    F = np.float32
    f64 = np.float64
    def fma(a, b, c):
        return F(f64(a) * f64(b) + f64(c))
    t = np.negative(x)
    q = fma(t, LOG2E, HALF)
    m = np.floor(q)
    r = F(t - F(CW1 * m))
    r = fma(m, C2P, r)
    p = fma(P0, r, P1)
    p = fma(p, r, P2)
    p = fma(p, r, P3)
    p = fma(p, r, P4)
    p = fma(p, r, HALF)
    r2 = F(r * r)
    y = fma(p, r2, r)
    y = F(F(1.0) + y)
    scale = ((m.astype(np.int32) + 127) << 23).view(np.float32)
    e = F(y * scale)
    d = F(e + F(1.0))
    return F(F(1.0) / d)
